# revision 18
# baseline (speedup 1.0000x reference)
"""BiLSTM (2-layer, H=50, D=207, T=30, B=16384) -> FC(2) Trainium2 kernel.

Data-parallel over 8 NeuronCores (2048 batch rows each). Host pre-packs
x into feature-major bf16 layout [T, 208, B] (ones row folds biases into
the input projection) and reorders weights into PSUM-bank gate layout.

v2: sigma-everywhere cell. All four gate banks go through ONE sigmoid
ACTIVATE: the g-gate weights are pre-doubled so bank g holds 2*g_pre and
sigma(2g) = (tanh(g)+1)/2. The cell carries c~ = 2c and h~ = h/2; every
h-consumer weight (Whh, Wih1, fc_w) is pre-doubled to compensate, so the
math stays exact:
    t  = (sg - 0.5) * si            # = i*tanh(g)/2      (stt, bf16 2x)
    c~ = sf * c~                    # in-place            (TT, f32)
    c~ = 4t + c~                                          (stt)
    sc = sigmoid(c~)                # = (tanh(c)+1)/2     (ACT)
    h~ = (sc - 0.5) * so            # = h/2               (stt, bf16 2x)
No tanh calls, 2 ACT calls/sweep instead of 3, G in bf16.

Device layout (per core, 4 sweeps of 512 batch):
  - quadrant packing: partitions 0:50 = dir-f (or sweep s1), 64:114 =
    dir-b (or sweep s2); weight columns zero-padded so junk rows are 0.
  - Phase A: layer-0 fwd+bwd scans in lockstep (fwd eats x[t=k], bwd
    x[t=29-k]); gates accumulate in PSUM [128, 4banks]; h~ -> state
    tile -> DMA into h1 history [115, 30, 512] (fwd store widened to
    rows 0:64 so junk rows 50:64 stay finite; ones row 114 via DMA).
  - Phase B: layer-1 forward, 2 sweeps quadrant-packed per matmul pair.
  - Phase C: layer-1 backward, only t=29 (first step of reverse scan is
    all the FC needs).
  - FC on device ([2, 512] psum per sweep); bias+transpose on host.
"""

import numpy as np
import ml_dtypes

import concourse.bass as bass
import concourse.tile as tile
from concourse import bacc, mybir
from concourse.bass_utils import run_bass_kernel_spmd

BF16 = mybir.dt.bfloat16
F32 = mybir.dt.float32
AF = mybir.ActivationFunctionType
ALU = mybir.AluOpType

H = 50
DIN = 207
DK = 208          # D + ones row
T = 30
B = 16384
NCORES = 8
BC = B // NCORES  # 2048
NSW = 4           # sweeps per core
BS = 512          # sweep batch size
Q = 64            # quadrant offset for second lane (dir-b / sweep s2)
KH = 114          # K rows for rec matmuls / h span (0:50 real, 64:114 real)
KH1 = 115         # K rows for L1 proj (incl. ones row at 114)

# PyTorch LSTM gate order in weight rows: i, f, g, o. Our bank order: i, f, o, g.
GATE_SLICES = [slice(0, 50), slice(50, 100), slice(150, 200), slice(100, 150)]
# per-bank pre-scale for the sigma-everywhere trick: bank 3 (g) doubled
GBANK = 3

USE_TP = __import__("os").environ.get("KERNEL_NO_TP", "0") != "1"  # col-tiled concurrent projection pairs


def _pack_weights(inp):
    f32 = np.float32
    # Scale plan (h~ = h/2 carried everywhere, c~ = 2c):
    #   L0 proj (input x, exact):        bank g x2, others x1
    #   L0 rec (input h~0):              x2, bank g x4
    #   L1 proj rows 0:114 (input h~0):  x2, bank g x4; bias row: x1 / g x2
    #   L1 rec (input h~1):              x2, bank g x4
    #   FC (input h~1):                  x2
    bank_s0 = [1.0, 1.0, 1.0, 2.0]  # exact-input projections, per bank
    # ---- L0 projection (+bias via ones row), [208, 4*128] ----
    w0 = np.zeros((DK, 512), f32)
    wf = np.concatenate([inp["wih0f"], inp["b0f"][:, None]], axis=1)  # [200,208]
    wb = np.concatenate([inp["wih0b"], inp["b0b"][:, None]], axis=1)
    for c, gs in enumerate(GATE_SLICES):
        w0[:, c * 128 + 0:c * 128 + 50] = bank_s0[c] * wf[gs].T
        w0[:, c * 128 + Q:c * 128 + Q + 50] = bank_s0[c] * wb[gs].T
    # ---- L0 recurrent, block-diag [114, 4*128] ----
    r0 = np.zeros((KH, 512), f32)
    for c, gs in enumerate(GATE_SLICES):
        s = 2.0 * bank_s0[c]
        r0[0:50, c * 128 + 0:c * 128 + 50] = s * inp["whh0f"][gs].T
        r0[Q:Q + 50, c * 128 + Q:c * 128 + Q + 50] = s * inp["whh0b"][gs].T
    # ---- L1 fwd projection [115, 4*64] (rows: h1f 0:50, h1b 64:114, bias 114)
    def l1_proj(wih, b):
        w = np.zeros((KH1, 256), f32)
        for c, gs in enumerate(GATE_SLICES):
            s = 2.0 * bank_s0[c]
            w[0:50, c * 64:c * 64 + 50] = s * wih[gs, 0:50].T
            w[Q:Q + 50, c * 64:c * 64 + 50] = s * wih[gs, 50:100].T
            w[KH, c * 64:c * 64 + 50] = bank_s0[c] * b[gs]
        return w
    w1 = l1_proj(inp["wih1f"], inp["b1f"])
    w1b = l1_proj(inp["wih1b"], inp["b1b"])
    # ---- L1 recurrent, block-diag per sweep pair [114, 4*128] ----
    r1 = np.zeros((KH, 512), f32)
    for c, gs in enumerate(GATE_SLICES):
        s = 2.0 * bank_s0[c]
        r1[0:50, c * 128 + 0:c * 128 + 50] = s * inp["whh1f"][gs].T
        r1[Q:Q + 50, c * 128 + Q:c * 128 + Q + 50] = s * inp["whh1f"][gs].T
    # ---- FC ----
    wff = np.ascontiguousarray(2.0 * inp["fc_w"][:, 0:50].T)  # [50, 2]
    wfb = np.ascontiguousarray(2.0 * inp["fc_w"][:, 50:100].T)
    bf = ml_dtypes.bfloat16
    return {
        "w0hi": w0[0:128].astype(bf), "w0lo": w0[128:DK].astype(bf),
        "r0": r0.astype(bf), "w1": w1.astype(bf), "w1b": w1b.astype(bf),
        "r1": r1.astype(bf), "wff": wff.astype(bf), "wfb": wfb.astype(bf),
        "ones": np.ones((1, T, 2 * BS), dtype=bf),
    }


def _pack_x(x):
    # x [B, T, 207] f32 -> [T, 208, B] bf16 with ones row at d=207
    xt = np.empty((T, DK, B), dtype=ml_dtypes.bfloat16)
    xt[:, 0:DIN, :] = x.transpose(1, 2, 0).astype(ml_dtypes.bfloat16)
    xt[:, DIN, :] = ml_dtypes.bfloat16(1.0)
    return xt


def _mm(nc, out, lhsT, rhs, start, stop, tp=None):
    kw = {}
    if tp is not None and USE_TP:
        kw["tile_position"] = tp
    nc.tensor.matmul(out, lhsT, rhs, start=start, stop=stop, **kw)


def _build_nc():
    nc = bacc.Bacc("TRN2", target_bir_lowering=False, debug=False)
    ap = {}
    ap["xT"] = nc.dram_tensor("xT", [T, DK, BC], BF16, kind="ExternalInput").ap()
    for name, shp in [("w0hi", [128, 512]), ("w0lo", [DK - 128, 512]),
                      ("r0", [KH, 512]), ("w1", [KH1, 256]), ("w1b", [KH1, 256]),
                      ("r1", [KH, 512]), ("wff", [50, 2]), ("wfb", [50, 2]),
                      ("ones", [1, T, 2 * BS])]:
        ap[name] = nc.dram_tensor(name, shp, BF16, kind="ExternalInput").ap()
    out_ap = nc.dram_tensor("out", [2, BC], F32, kind="ExternalOutput").ap()

    with tile.TileContext(nc) as tc:
        with (
            tc.tile_pool(name="wts", bufs=1) as wp,
            tc.tile_pool(name="xin", bufs=2) as xp,
            tc.tile_pool(name="h1p", bufs=1) as h1p,
            tc.tile_pool(name="st", bufs=1) as sp,
            tc.tile_pool(name="gt", bufs=1) as gp,
            tc.tile_pool(name="outp", bufs=1) as op_,
        ):
            # ---- load weights to SBUF ----
            w = {}
            for name, shp in [("w0hi", [128, 512]), ("w0lo", [DK - 128, 512]),
                              ("r0", [KH, 512]), ("w1", [KH1, 256]),
                              ("w1b", [KH1, 256]), ("r1", [KH, 512]),
                              ("wff", [50, 2]), ("wfb", [50, 2])]:
                t = wp.tile(shp, BF16, tag=name, name=name)
                nc.sync.dma_start(t[:], ap[name][:])
                w[name] = t
            # FC weights staged at both quadrants (rhs base-partition match)
            for name in ("wff", "wfb"):
                t = wp.tile([128, 2], BF16, tag=name + "q", name=name + "q")
                nc.sync.dma_start(t[0:50, :], ap[name][:])
                nc.sync.dma_start(t[Q:Q + 50, :], ap[name][:])
                w[name + "q"] = t

            # ---- h1 history per sweep-PAIR [115, T, 1024]; row 114 = ones
            # (rows 50:64 get finite junk from the widened fwd h1 DMA each
            # step; row 114 is the L1 bias ones row, written once by DMA;
            # rows 115:128 are outside the K span.)
            h1 = []
            for p in range(2):
                t = h1p.tile([128, T, 2 * BS], BF16, tag=f"h1_{p}", name=f"h1_{p}")
                nc.sync.dma_start(t[KH:KH + 1, :, :], ap["ones"][:])
                h1.append(t)

            # ---- states (h~ per sweep-PAIR so h1 stores are 1 DMA/pair) ----
            hS = [sp.tile([128, 2 * BS], BF16, tag=f"hs{p}", name=f"hs{p}") for p in range(2)]
            cS = [sp.tile([128, BS], F32, tag=f"cs{s}", name=f"cs{s}") for s in range(NSW)]
            tS = [sp.tile([128, BS], BF16, tag=f"ts{s}", name=f"ts{s}") for s in range(NSW)]
            # G in F32: (sg-0.5)/(sc-0.5) would cancel catastrophically in
            # bf16 near g=0/c=0; fp32 sigma outputs keep the subtraction
            # exact, bf16 only on t/h~ where error is relative.
            hB = [sp.tile([128, BS], BF16, tag=f"hb{p}", name=f"hb{p}") for p in range(2)]
            cB = [sp.tile([128, BS], F32, tag=f"cb{p}", name=f"cb{p}") for p in range(2)]
            tB = [sp.tile([128, BS], BF16, tag=f"tb{p}", name=f"tb{p}") for p in range(2)]
            hC = [sp.tile([128, BS], BF16, tag=f"hc{p}", name=f"hc{p}") for p in range(2)]
            cC = [sp.tile([128, BS], F32, tag=f"cc{p}", name=f"cc{p}") for p in range(2)]
            tC = [sp.tile([128, BS], BF16, tag=f"tc{p}", name=f"tc{p}") for p in range(2)]

            def cell(P, G, t_, c_t, h_out, k):
                """P: psum [128,4,BS] gates i,f,o,g2; G: sbuf f32 [128,4,BS];
                t_: bf16 [128,BS] scratch; c_t: [128,BS] f32 (holds c~=2c);
                h_out: [128,BS] bf16 dest (rows 0:KH, holds h~=h/2).
                sc = sigma(c~) reuses G slot 0 (si is dead by then).
                The post-sigmoid tail is priority-hoisted so sigma(c~)/h~
                don't queue behind the next sweep's big 4-bank sigmoid."""
                nc.scalar.activation(G[0:KH, :, :], P[0:KH, :, :], AF.Sigmoid)
                si, sf, so, sg = (G[0:KH, j, :] for j in range(4))
                tt = t_[0:KH, :]
                with tc.high_priority(offset=60):
                    # t = (sg - 0.5) * si  == i * tanh(g) / 2
                    nc.vector.scalar_tensor_tensor(tt, sg, 0.5, si,
                                                   ALU.subtract, ALU.mult)
                    if k == 0:
                        nc.vector.tensor_scalar_mul(c_t[0:KH, :], tt, 4.0)
                    else:
                        # c~ *= sigma(f) on the otherwise-idle GPSIMD: runs
                        # concurrently with the t stt on DVE, shortening the
                        # serial cell chain and cutting DVE busy by ~25%.
                        nc.gpsimd.tensor_mul(c_t[0:KH, :], sf, c_t[0:KH, :])
                        nc.vector.scalar_tensor_tensor(c_t[0:KH, :], tt, 4.0,
                                                       c_t[0:KH, :],
                                                       ALU.mult, ALU.add)
                    # sc = sigma(c~) = (tanh(c)+1)/2 -> reuse si slot
                    nc.scalar.activation(si, c_t[0:KH, :], AF.Sigmoid)
                    # h~ = (sc - 0.5) * so == o * tanh(c) / 2
                    nc.vector.scalar_tensor_tensor(h_out[0:KH, :], si, 0.5, so,
                                                   ALU.subtract, ALU.mult)

            with tc.tile_pool(name="ps", bufs=2, space="PSUM") as pp:
                # =================== Phase A: layer 0, lockstep ===========
                for k in range(T):
                    for p in range(2):
                        psl = bass.ts(p, 2 * BS)
                        # pair-wide x tiles (2 sweeps per DMA): 8 Sync
                        # issues/step instead of 16 - the DMA-issue path on
                        # the Sync queue (~0.7us per DMA_DIRECT2D) was the
                        # phase-A co-bottleneck.
                        xfh = xp.tile([128, 2 * BS], BF16, tag="xfh", name="xfh")
                        nc.sync.dma_start(xfh[:], ap["xT"][k, 0:128, psl])
                        xfl = xp.tile([DK - 128, 2 * BS], BF16, tag="xfl", name="xfl")
                        nc.sync.dma_start(xfl[:], ap["xT"][k, 128:DK, psl])
                        xbh = xp.tile([128, 2 * BS], BF16, tag="xbh", name="xbh")
                        nc.sync.dma_start(xbh[:], ap["xT"][T - 1 - k, 0:128, psl])
                        xbl = xp.tile([DK - 128, 2 * BS], BF16, tag="xbl", name="xbl")
                        nc.sync.dma_start(xbl[:], ap["xT"][T - 1 - k, 128:DK, psl])
                        for j in range(2):
                            s = 2 * p + j
                            sl = bass.ts(j, BS)

                            # Rotated interleave: adjacent MMs hit disjoint
                            # col-groups (concurrent) while same-bank writes
                            # are >=2 slots apart (concurrent drains from
                            # different col-groups into one bank corrupt it).
                            P = pp.tile([128, 4, BS], F32, tag="ps", name="ps")
                            for wt_, xfx, xbx, st in ((w["w0hi"], xfh, xbh, True),
                                                      (w["w0lo"], xfl, xbl, False)):
                                for c in range(4):
                                    cb = (c + 2) % 4
                                    spA = k == 0 and not st and c >= 2
                                    spB = k == 0 and not st and cb < 2
                                    _mm(nc, P[0:Q, c, :],
                                        wt_[:, bass.ds(c * 128, Q)], xfx[:, sl],
                                        start=st, stop=spA)
                                    _mm(nc, P[Q:128, cb, :],
                                        wt_[:, bass.ds(cb * 128 + Q, Q)], xbx[:, sl],
                                        start=st, stop=spB)
                            if k > 0:
                                for c in range(4):
                                    _mm(nc, P[:, c, :],
                                        w["r0"][:, bass.ts(c, 128)],
                                        hS[p][0:KH, sl],
                                        start=False, stop=True)

                            G = gp.tile([128, 4, BS], F32, tag=f"g{s}", name=f"g{s}")
                            cell(P, G, tS[s], cS[s], hS[p][:, sl], k)
                        # h1 history is time-aligned: bwd lane at step k
                        # holds h_b for time T-1-k. Fwd store widened to
                        # 0:64 so junk rows 50:64 hold finite data. One
                        # DMA per pair (both sweeps' h~ contiguous in hS).
                        nc.sync.dma_start(h1[p][0:Q, k, :], hS[p][0:Q, :])
                        nc.sync.dma_start(h1[p][Q:Q + 50, T - 1 - k, :],
                                          hS[p][Q:Q + 50, :])

                # =================== Phase B: layer 1 forward =============
                for k in range(T):
                    for p in range(2):
                        h1a = h1[p][0:KH1, k, 0:BS]
                        h1b = h1[p][0:KH1, k, BS:2 * BS]
                        P = pp.tile([128, 4, BS], F32, tag="ps", name="ps")
                        for c in range(4):
                            cb = (c + 2) % 4
                            _mm(nc, P[0:Q, c, :], w["w1"][:, bass.ts(c, 64)],
                                h1a, start=True,
                                stop=(k == 0 and c >= 2))
                            _mm(nc, P[Q:128, cb, :], w["w1"][:, bass.ts(cb, 64)],
                                h1b, start=True,
                                stop=(k == 0 and cb < 2))
                        if k > 0:
                            for c in range(4):
                                _mm(nc, P[:, c, :],
                                    w["r1"][:, bass.ts(c, 128)], hB[p][0:KH, :],
                                    start=False, stop=True)
                        G = gp.tile([128, 4, BS], F32, tag=f"g{p}", name=f"g{p}")
                        cell(P, G, tB[p], cB[p], hB[p], k)

                # =================== Phase C: layer 1 backward (t=29) =====
                for p in range(2):
                    h1a = h1[p][0:KH1, T - 1, 0:BS]
                    h1b = h1[p][0:KH1, T - 1, BS:2 * BS]
                    P = pp.tile([128, 4, BS], F32, tag="ps", name="ps")
                    for c in range(4):
                        cb = (c + 2) % 4
                        _mm(nc, P[0:Q, c, :], w["w1b"][:, bass.ts(c, 64)],
                            h1a, start=True, stop=c >= 2)
                        _mm(nc, P[Q:128, cb, :], w["w1b"][:, bass.ts(cb, 64)],
                            h1b, start=True, stop=cb < 2)
                    G = gp.tile([128, 4, BS], F32, tag=f"g{p}", name=f"g{p}")
                    cell(P, G, tC[p], cC[p], hC[p], 0)

            # =================== FC ====================================
            with tc.tile_pool(name="fcp", bufs=4, space="PSUM") as fcp:
                for s in range(NSW):
                    p, qo = s // 2, (s % 2) * Q
                    F = fcp.tile([2, BS], F32, tag="fc", name="fc")
                    nc.tensor.matmul(F[:], w["wffq"][qo:qo + 50, :],
                                     hB[p][qo:qo + 50, :], start=True, stop=False)
                    nc.tensor.matmul(F[:], w["wfbq"][qo:qo + 50, :],
                                     hC[p][qo:qo + 50, :], start=False, stop=True)
                    ot = op_.tile([2, BS], F32, tag="o", name="ot")
                    nc.vector.tensor_copy(ot[:], F[:])
                    nc.sync.dma_start(out_ap[:, bass.ts(s, BS)], ot[:])

    nc.compile()
    return nc


_NC_CACHE = None


def kernel(**inputs) -> np.ndarray:
    global _NC_CACHE
    if _NC_CACHE is None:
        _NC_CACHE = _build_nc()
    nc = _NC_CACHE
    wts = _pack_weights(inputs)
    xt = _pack_x(np.asarray(inputs["x"], dtype=np.float32))
    in_maps = []
    for c in range(NCORES):
        m = dict(wts)
        m["xT"] = np.ascontiguousarray(xt[:, :, c * BC:(c + 1) * BC])
        in_maps.append(m)
    res = run_bass_kernel_spmd(nc, in_maps, list(range(NCORES)))
    outs = [res.results[c]["out"] for c in range(NCORES)]  # [2, BC] each
    full = np.concatenate(outs, axis=1).T  # [B, 2]
    return (full + inputs["fc_b"][None, :]).astype(np.float32)


# revision 23
# speedup vs baseline: 1.3197x; 1.3197x over previous
"""BiLSTM (2-layer, H=50, D=207, T=30, B=16384) -> FC(2) Trainium2 kernel.

Data-parallel over 8 NeuronCores (2048 batch rows each). Host pre-packs
x into feature-major bf16 layout [T, 208, B] (ones row folds biases into
the input projection) and reorders weights into PSUM-bank gate layout.

v2: sigma-everywhere cell. All four gate banks go through ONE sigmoid
ACTIVATE: the g-gate weights are pre-doubled so bank g holds 2*g_pre and
sigma(2g) = (tanh(g)+1)/2. The cell carries c~ = 2c and h~ = h/2; every
h-consumer weight (Whh, Wih1, fc_w) is pre-doubled to compensate, so the
math stays exact:
    t  = (sg - 0.5) * si            # = i*tanh(g)/2      (stt, bf16 2x)
    c~ = sf * c~                    # in-place            (TT, f32)
    c~ = 4t + c~                                          (stt)
    sc = sigmoid(c~)                # = (tanh(c)+1)/2     (ACT)
    h~ = (sc - 0.5) * so            # = h/2               (stt, bf16 2x)
No tanh calls, 2 ACT calls/sweep instead of 3, G in bf16.

Device layout (per core, 4 sweeps of 512 batch):
  - quadrant packing: partitions 0:50 = dir-f (or sweep s1), 64:114 =
    dir-b (or sweep s2); weight columns zero-padded so junk rows are 0.
  - Phase A: layer-0 fwd+bwd scans in lockstep (fwd eats x[t=k], bwd
    x[t=29-k]); gates accumulate in PSUM [128, 4banks]; h~ -> state
    tile -> DMA into h1 history [115, 30, 512] (fwd store widened to
    rows 0:64 so junk rows 50:64 stay finite; ones row 114 via DMA).
  - Phase B: layer-1 forward, 2 sweeps quadrant-packed per matmul pair.
  - Phase C: layer-1 backward, only t=29 (first step of reverse scan is
    all the FC needs).
  - FC on device ([2, 512] psum per sweep); bias+transpose on host.
"""

import numpy as np
import ml_dtypes

import concourse.bass as bass
import concourse.tile as tile
from concourse import bacc, mybir
from concourse.bass_utils import run_bass_kernel_spmd

BF16 = mybir.dt.bfloat16
F32 = mybir.dt.float32
AF = mybir.ActivationFunctionType
ALU = mybir.AluOpType

H = 50
DIN = 207
DK = 208          # D + ones row
T = 30
B = 16384
NCORES = 8
BC = B // NCORES  # 2048
NSW = 4           # sweeps per core
BS = 512          # sweep batch size
Q = 64            # quadrant offset for second lane (dir-b / sweep s2)
KH = 114          # K rows for rec matmuls / h span (0:50 real, 64:114 real)
KH1 = 115         # K rows for L1 proj (incl. ones row at 114)

# PyTorch LSTM gate order in weight rows: i, f, g, o. Our bank order: i, f, o, g.
GATE_SLICES = [slice(0, 50), slice(50, 100), slice(150, 200), slice(100, 150)]
# per-bank pre-scale for the sigma-everywhere trick: bank 3 (g) doubled
GBANK = 3

USE_TP = __import__("os").environ.get("KERNEL_NO_TP", "0") != "1"  # col-tiled concurrent projection pairs


def _pack_weights(inp):
    f32 = np.float32
    # Scale plan (h~ = h/2 carried everywhere, c~ = 2c):
    #   L0 proj (input x, exact):        bank g x2, others x1
    #   L0 rec (input h~0):              x2, bank g x4
    #   L1 proj rows 0:114 (input h~0):  x2, bank g x4; bias row: x1 / g x2
    #   L1 rec (input h~1):              x2, bank g x4
    #   FC (input h~1):                  x2
    bank_s0 = [1.0, 1.0, 1.0, 2.0]  # exact-input projections, per bank
    # ---- L0 projection (+bias via ones row), [208, 4*128] ----
    w0 = np.zeros((DK, 512), f32)
    wf = np.concatenate([inp["wih0f"], inp["b0f"][:, None]], axis=1)  # [200,208]
    wb = np.concatenate([inp["wih0b"], inp["b0b"][:, None]], axis=1)
    for c, gs in enumerate(GATE_SLICES):
        w0[:, c * 128 + 0:c * 128 + 50] = bank_s0[c] * wf[gs].T
        w0[:, c * 128 + Q:c * 128 + Q + 50] = bank_s0[c] * wb[gs].T
    # ---- L0 recurrent, block-diag [114, 4*128] ----
    r0 = np.zeros((KH, 512), f32)
    for c, gs in enumerate(GATE_SLICES):
        s = 2.0 * bank_s0[c]
        r0[0:50, c * 128 + 0:c * 128 + 50] = s * inp["whh0f"][gs].T
        r0[Q:Q + 50, c * 128 + Q:c * 128 + Q + 50] = s * inp["whh0b"][gs].T
    # ---- L1 fwd projection [115, 4*64] (rows: h1f 0:50, h1b 64:114, bias 114)
    def l1_proj(wih, b):
        w = np.zeros((KH1, 256), f32)
        for c, gs in enumerate(GATE_SLICES):
            s = 2.0 * bank_s0[c]
            w[0:50, c * 64:c * 64 + 50] = s * wih[gs, 0:50].T
            w[Q:Q + 50, c * 64:c * 64 + 50] = s * wih[gs, 50:100].T
            w[KH, c * 64:c * 64 + 50] = bank_s0[c] * b[gs]
        return w
    w1 = l1_proj(inp["wih1f"], inp["b1f"])
    w1b = l1_proj(inp["wih1b"], inp["b1b"])
    # ---- L1 recurrent, block-diag per sweep pair [114, 4*128] ----
    r1 = np.zeros((KH, 512), f32)
    for c, gs in enumerate(GATE_SLICES):
        s = 2.0 * bank_s0[c]
        r1[0:50, c * 128 + 0:c * 128 + 50] = s * inp["whh1f"][gs].T
        r1[Q:Q + 50, c * 128 + Q:c * 128 + Q + 50] = s * inp["whh1f"][gs].T
    # ---- FC ----
    wff = np.ascontiguousarray(2.0 * inp["fc_w"][:, 0:50].T)  # [50, 2]
    wfb = np.ascontiguousarray(2.0 * inp["fc_w"][:, 50:100].T)
    bf = ml_dtypes.bfloat16
    return {
        "w0hi": w0[0:128].astype(bf), "w0lo": w0[128:DK].astype(bf),
        "r0": r0.astype(bf), "w1": w1.astype(bf), "w1b": w1b.astype(bf),
        "r1": r1.astype(bf), "wff": wff.astype(bf), "wfb": wfb.astype(bf),
        "ones": np.ones((1, T, 2 * BS), dtype=bf),
    }


def _pack_x(x):
    # x [B, T, 207] f32 -> [T, 208, B] bf16 with ones row at d=207
    xt = np.empty((T, DK, B), dtype=ml_dtypes.bfloat16)
    xt[:, 0:DIN, :] = x.transpose(1, 2, 0).astype(ml_dtypes.bfloat16)
    xt[:, DIN, :] = ml_dtypes.bfloat16(1.0)
    return xt


def _mm(nc, out, lhsT, rhs, start, stop, tp=None):
    kw = {}
    if tp is not None and USE_TP:
        kw["tile_position"] = tp
    nc.tensor.matmul(out, lhsT, rhs, start=start, stop=stop, **kw)


def _build_nc():
    nc = bacc.Bacc("TRN2", target_bir_lowering=False, debug=False)
    ap = {}
    ap["xT"] = nc.dram_tensor("xT", [T, DK, BC], BF16, kind="ExternalInput").ap()
    for name, shp in [("w0hi", [128, 512]), ("w0lo", [DK - 128, 512]),
                      ("r0", [KH, 512]), ("w1", [KH1, 256]), ("w1b", [KH1, 256]),
                      ("r1", [KH, 512]), ("wff", [50, 2]), ("wfb", [50, 2]),
                      ("ones", [1, T, 2 * BS])]:
        ap[name] = nc.dram_tensor(name, shp, BF16, kind="ExternalInput").ap()
    out_ap = nc.dram_tensor("out", [2, BC], F32, kind="ExternalOutput").ap()

    with tile.TileContext(nc) as tc:
        with (
            tc.tile_pool(name="wts", bufs=1) as wp,
            tc.tile_pool(name="xin", bufs=2) as xp,
            tc.tile_pool(name="h1p", bufs=1) as h1p,
            tc.tile_pool(name="st", bufs=1) as sp,
            tc.tile_pool(name="gt", bufs=1) as gp,
            tc.tile_pool(name="outp", bufs=1) as op_,
        ):
            # ---- load weights to SBUF ----
            w = {}
            for name, shp in [("w0hi", [128, 512]), ("w0lo", [DK - 128, 512]),
                              ("r0", [KH, 512]), ("w1", [KH1, 256]),
                              ("w1b", [KH1, 256]), ("r1", [KH, 512]),
                              ("wff", [50, 2]), ("wfb", [50, 2])]:
                t = wp.tile(shp, BF16, tag=name, name=name)
                nc.sync.dma_start(t[:], ap[name][:])
                w[name] = t
            # FC weights staged at both quadrants (rhs base-partition match)
            for name in ("wff", "wfb"):
                t = wp.tile([128, 2], BF16, tag=name + "q", name=name + "q")
                nc.sync.dma_start(t[0:50, :], ap[name][:])
                nc.sync.dma_start(t[Q:Q + 50, :], ap[name][:])
                w[name + "q"] = t

            # ---- h1 history per sweep-PAIR [115, T, 1024]; row 114 = ones
            # (rows 50:64 get finite junk from the widened fwd h1 DMA each
            # step; row 114 is the L1 bias ones row, written once by DMA;
            # rows 115:128 are outside the K span.)
            h1 = []
            for p in range(2):
                t = h1p.tile([128, T, 2 * BS], BF16, tag=f"h1_{p}", name=f"h1_{p}")
                nc.sync.dma_start(t[KH:KH + 1, :, :], ap["ones"][:])
                h1.append(t)

            # ---- states (h~/c~ per sweep-PAIR so h1 stores are 1 DMA/pair
            # and sigma(c~) is one ACT call per pair) ----
            hS = [sp.tile([128, 2 * BS], BF16, tag=f"hs{p}", name=f"hs{p}") for p in range(2)]
            cS = [sp.tile([128, 2, BS], F32, tag=f"cs{p}", name=f"cs{p}") for p in range(2)]
            tS = [sp.tile([128, BS], BF16, tag=f"ts{s}", name=f"ts{s}") for s in range(NSW)]
            # G in F32: (sg-0.5)/(sc-0.5) would cancel catastrophically in
            # bf16 near g=0/c=0; fp32 sigma outputs keep the subtraction
            # exact, bf16 only on t/h~ where error is relative.
            hB = [sp.tile([128, BS], BF16, tag=f"hb{p}", name=f"hb{p}") for p in range(2)]
            cB = [sp.tile([128, BS], F32, tag=f"cb{p}", name=f"cb{p}") for p in range(2)]
            tB = [sp.tile([128, BS], BF16, tag=f"tb{p}", name=f"tb{p}") for p in range(2)]
            hC = [sp.tile([128, BS], BF16, tag=f"hc{p}", name=f"hc{p}") for p in range(2)]
            cC = [sp.tile([128, BS], F32, tag=f"cc{p}", name=f"cc{p}") for p in range(2)]
            tC = [sp.tile([128, BS], BF16, tag=f"tc{p}", name=f"tc{p}") for p in range(2)]

            def cell(P, G, t_, c_t, h_out, k):
                """P: psum [128,4,BS] gates i,f,o,g2; G: sbuf f32 [128,4,BS];
                t_: bf16 [128,BS] scratch; c_t: [128,BS] f32 (holds c~=2c);
                h_out: [128,BS] bf16 dest (rows 0:KH, holds h~=h/2).
                sc = sigma(c~) reuses G slot 0 (si is dead by then).
                The post-sigmoid tail is priority-hoisted so sigma(c~)/h~
                don't queue behind the next sweep's big 4-bank sigmoid."""
                nc.scalar.activation(G[0:KH, :, :], P[0:KH, :, :], AF.Sigmoid)
                si, sf, so, sg = (G[0:KH, j, :] for j in range(4))
                tt = t_[0:KH, :]
                with tc.high_priority(offset=60):
                    # t = (sg - 0.5) * si  == i * tanh(g) / 2
                    nc.vector.scalar_tensor_tensor(tt, sg, 0.5, si,
                                                   ALU.subtract, ALU.mult)
                    if k == 0:
                        nc.vector.tensor_scalar_mul(c_t[0:KH, :], tt, 4.0)
                    else:
                        nc.vector.tensor_mul(c_t[0:KH, :], sf, c_t[0:KH, :])
                        nc.vector.scalar_tensor_tensor(c_t[0:KH, :], tt, 4.0,
                                                       c_t[0:KH, :],
                                                       ALU.mult, ALU.add)
                    # sc = sigma(c~) = (tanh(c)+1)/2 -> reuse si slot
                    nc.scalar.activation(si, c_t[0:KH, :], AF.Sigmoid)
                    # h~ = (sc - 0.5) * so == o * tanh(c) / 2
                    nc.vector.scalar_tensor_tensor(h_out[0:KH, :], si, 0.5, so,
                                                   ALU.subtract, ALU.mult)

            with tc.tile_pool(name="ps", bufs=2, space="PSUM") as pp:
                # =================== Phase A: layer 0, lockstep ===========
                for k in range(T):
                    for p in range(2):
                        psl = bass.ts(p, 2 * BS)
                        # pair-wide x tiles (2 sweeps per DMA): 8 Sync
                        # issues/step instead of 16 - the DMA-issue path on
                        # the Sync queue (~0.7us per DMA_DIRECT2D) was the
                        # phase-A co-bottleneck.
                        xfh = xp.tile([128, 2 * BS], BF16, tag="xfh", name="xfh")
                        nc.sync.dma_start(xfh[:], ap["xT"][k, 0:128, psl])
                        xfl = xp.tile([DK - 128, 2 * BS], BF16, tag="xfl", name="xfl")
                        nc.sync.dma_start(xfl[:], ap["xT"][k, 128:DK, psl])
                        xbh = xp.tile([128, 2 * BS], BF16, tag="xbh", name="xbh")
                        nc.sync.dma_start(xbh[:], ap["xT"][T - 1 - k, 0:128, psl])
                        xbl = xp.tile([DK - 128, 2 * BS], BF16, tag="xbl", name="xbl")
                        nc.sync.dma_start(xbl[:], ap["xT"][T - 1 - k, 128:DK, psl])
                        for j in range(2):
                            s = 2 * p + j
                            sl = bass.ts(j, BS)

                            # Rotated interleave: adjacent MMs hit disjoint
                            # col-groups (concurrent) while same-bank writes
                            # are >=2 slots apart (concurrent drains from
                            # different col-groups into one bank corrupt it).
                            P = pp.tile([128, 4, BS], F32, tag="ps", name="ps")
                            for wt_, xfx, xbx, st in ((w["w0hi"], xfh, xbh, True),
                                                      (w["w0lo"], xfl, xbl, False)):
                                for c in range(4):
                                    cb = (c + 2) % 4
                                    spA = k == 0 and not st and c >= 2
                                    spB = k == 0 and not st and cb < 2
                                    _mm(nc, P[0:Q, c, :],
                                        wt_[:, bass.ds(c * 128, Q)], xfx[:, sl],
                                        start=st, stop=spA)
                                    _mm(nc, P[Q:128, cb, :],
                                        wt_[:, bass.ds(cb * 128 + Q, Q)], xbx[:, sl],
                                        start=st, stop=spB)
                            if k > 0:
                                for c in range(4):
                                    _mm(nc, P[:, c, :],
                                        w["r0"][:, bass.ts(c, 128)],
                                        hS[p][0:KH, sl],
                                        start=False, stop=True)

                            # pair-G [128, 2(sweep), 4(gate), BS]: sigma
                            # outputs for both sweeps of the pair, so the
                            # sigma(c~) for the pair is ONE ACT call.
                            if j == 0:
                                Gp = gp.tile([128, 2, 4, BS], F32,
                                             tag=f"g{p}", name=f"g{p}")
                            G = Gp[:, j, :, :]
                            nc.scalar.activation(G[0:KH, :, :], P[0:KH, :, :],
                                                 AF.Sigmoid)
                            si, sf, so, sg = (G[0:KH, q, :] for q in range(4))
                            tt = tS[s][0:KH, :]
                            cj = cS[p][0:KH, j, :]
                            with tc.high_priority(offset=60):
                                # t = (sg - 0.5) * si  == i * tanh(g) / 2
                                nc.vector.scalar_tensor_tensor(
                                    tt, sg, 0.5, si, ALU.subtract, ALU.mult)
                                if k == 0:
                                    nc.vector.tensor_scalar_mul(cj, tt, 4.0)
                                else:
                                    nc.vector.tensor_mul(cj, sf, cj)
                                    nc.vector.scalar_tensor_tensor(
                                        cj, tt, 4.0, cj, ALU.mult, ALU.add)
                        with tc.high_priority(offset=60):
                            # sc = sigma(c~) for BOTH sweeps in one call;
                            # lands in the dead sg slots.
                            nc.scalar.activation(Gp[0:KH, :, 3, :],
                                                 cS[p][0:KH, :, :], AF.Sigmoid)
                            for j in range(2):
                                sl = bass.ts(j, BS)
                                # h~ = (sc - 0.5) * so == o * tanh(c) / 2
                                nc.vector.scalar_tensor_tensor(
                                    hS[p][0:KH, sl], Gp[0:KH, j, 3, :], 0.5,
                                    Gp[0:KH, j, 2, :], ALU.subtract, ALU.mult)
                        # h1 history is time-aligned: bwd lane at step k
                        # holds h_b for time T-1-k. Fwd store widened to
                        # 0:64 so junk rows 50:64 hold finite data. One
                        # DMA per pair (both sweeps' h~ contiguous in hS).
                        nc.sync.dma_start(h1[p][0:Q, k, :], hS[p][0:Q, :])
                        nc.sync.dma_start(h1[p][Q:Q + 50, T - 1 - k, :],
                                          hS[p][Q:Q + 50, :])

                # =================== Phase B: layer 1 forward =============
                for k in range(T):
                    for p in range(2):
                        h1a = h1[p][0:KH1, k, 0:BS]
                        h1b = h1[p][0:KH1, k, BS:2 * BS]
                        P = pp.tile([128, 4, BS], F32, tag="ps", name="ps")
                        for c in range(4):
                            cb = (c + 2) % 4
                            _mm(nc, P[0:Q, c, :], w["w1"][:, bass.ts(c, 64)],
                                h1a, start=True,
                                stop=(k == 0 and c >= 2))
                            _mm(nc, P[Q:128, cb, :], w["w1"][:, bass.ts(cb, 64)],
                                h1b, start=True,
                                stop=(k == 0 and cb < 2))
                        if k > 0:
                            for c in range(4):
                                _mm(nc, P[:, c, :],
                                    w["r1"][:, bass.ts(c, 128)], hB[p][0:KH, :],
                                    start=False, stop=True)
                        G2 = gp.tile([128, 2, 4, BS], F32, tag=f"g{p}", name=f"g{p}")
                        cell(P, G2[:, 0, :, :], tB[p], cB[p], hB[p], k)

                # =================== Phase C: layer 1 backward (t=29) =====
                for p in range(2):
                    h1a = h1[p][0:KH1, T - 1, 0:BS]
                    h1b = h1[p][0:KH1, T - 1, BS:2 * BS]
                    P = pp.tile([128, 4, BS], F32, tag="ps", name="ps")
                    for c in range(4):
                        cb = (c + 2) % 4
                        _mm(nc, P[0:Q, c, :], w["w1b"][:, bass.ts(c, 64)],
                            h1a, start=True, stop=c >= 2)
                        _mm(nc, P[Q:128, cb, :], w["w1b"][:, bass.ts(cb, 64)],
                            h1b, start=True, stop=cb < 2)
                    G2 = gp.tile([128, 2, 4, BS], F32, tag=f"g{p}", name=f"g{p}")
                    cell(P, G2[:, 1, :, :], tC[p], cC[p], hC[p], 0)

            # =================== FC ====================================
            with tc.tile_pool(name="fcp", bufs=4, space="PSUM") as fcp:
                for s in range(NSW):
                    p, qo = s // 2, (s % 2) * Q
                    F = fcp.tile([2, BS], F32, tag="fc", name="fc")
                    nc.tensor.matmul(F[:], w["wffq"][qo:qo + 50, :],
                                     hB[p][qo:qo + 50, :], start=True, stop=False)
                    nc.tensor.matmul(F[:], w["wfbq"][qo:qo + 50, :],
                                     hC[p][qo:qo + 50, :], start=False, stop=True)
                    ot = op_.tile([2, BS], F32, tag="o", name="ot")
                    nc.vector.tensor_copy(ot[:], F[:])
                    nc.sync.dma_start(out_ap[:, bass.ts(s, BS)], ot[:])

    nc.compile()
    return nc


_NC_CACHE = None


def kernel(**inputs) -> np.ndarray:
    global _NC_CACHE
    if _NC_CACHE is None:
        _NC_CACHE = _build_nc()
    nc = _NC_CACHE
    wts = _pack_weights(inputs)
    xt = _pack_x(np.asarray(inputs["x"], dtype=np.float32))
    in_maps = []
    for c in range(NCORES):
        m = dict(wts)
        m["xT"] = np.ascontiguousarray(xt[:, :, c * BC:(c + 1) * BC])
        in_maps.append(m)
    res = run_bass_kernel_spmd(nc, in_maps, list(range(NCORES)))
    outs = [res.results[c]["out"] for c in range(NCORES)]  # [2, BC] each
    full = np.concatenate(outs, axis=1).T  # [B, 2]
    return (full + inputs["fc_b"][None, :]).astype(np.float32)


# revision 25
# speedup vs baseline: 1.5011x; 1.1375x over previous
"""BiLSTM (2-layer, H=50, D=207, T=30, B=16384) -> FC(2) Trainium2 kernel.

Data-parallel over 8 NeuronCores (2048 batch rows each). Host pre-packs
x into feature-major bf16 layout [T, 208, B] (ones row folds biases into
the input projection) and reorders weights into PSUM-bank gate layout.

v2: sigma-everywhere cell. All four gate banks go through ONE sigmoid
ACTIVATE: the g-gate weights are pre-doubled so bank g holds 2*g_pre and
sigma(2g) = (tanh(g)+1)/2. The cell carries c~ = 2c and h~ = h/2; every
h-consumer weight (Whh, Wih1, fc_w) is pre-doubled to compensate, so the
math stays exact:
    t  = (sg - 0.5) * si            # = i*tanh(g)/2      (stt, bf16 2x)
    c~ = sf * c~                    # in-place            (TT, f32)
    c~ = 4t + c~                                          (stt)
    sc = sigmoid(c~)                # = (tanh(c)+1)/2     (ACT)
    h~ = (sc - 0.5) * so            # = h/2               (stt, bf16 2x)
No tanh calls, 2 ACT calls/sweep instead of 3, G in bf16.

Device layout (per core, 4 sweeps of 512 batch):
  - quadrant packing: partitions 0:50 = dir-f (or sweep s1), 64:114 =
    dir-b (or sweep s2); weight columns zero-padded so junk rows are 0.
  - Phase A: layer-0 fwd+bwd scans in lockstep (fwd eats x[t=k], bwd
    x[t=29-k]); gates accumulate in PSUM [128, 4banks]; h~ -> state
    tile -> DMA into h1 history [115, 30, 512] (fwd store widened to
    rows 0:64 so junk rows 50:64 stay finite; ones row 114 via DMA).
  - Phase B: layer-1 forward, 2 sweeps quadrant-packed per matmul pair.
  - Phase C: layer-1 backward, only t=29 (first step of reverse scan is
    all the FC needs).
  - FC on device ([2, 512] psum per sweep); bias+transpose on host.
"""

import numpy as np
import ml_dtypes

import concourse.bass as bass
import concourse.tile as tile
from concourse import bacc, mybir
from concourse.bass_utils import run_bass_kernel_spmd

BF16 = mybir.dt.bfloat16
F32 = mybir.dt.float32
AF = mybir.ActivationFunctionType
ALU = mybir.AluOpType

H = 50
DIN = 207
DK = 208          # D + ones row
T = 30
B = 16384
NCORES = 8
BC = B // NCORES  # 2048
NSW = 4           # sweeps per core
BS = 512          # sweep batch size
Q = 64            # quadrant offset for second lane (dir-b / sweep s2)
KH = 114          # K rows for rec matmuls / h span (0:50 real, 64:114 real)
KH1 = 115         # K rows for L1 proj (incl. ones row at 114)

# PyTorch LSTM gate order in weight rows: i, f, g, o. Our bank order: i, f, o, g.
GATE_SLICES = [slice(0, 50), slice(50, 100), slice(150, 200), slice(100, 150)]
# per-bank pre-scale for the sigma-everywhere trick: bank 3 (g) doubled
GBANK = 3

USE_TP = __import__("os").environ.get("KERNEL_NO_TP", "0") != "1"  # col-tiled concurrent projection pairs


def _pack_weights(inp):
    f32 = np.float32
    # Scale plan (h~ = h/2 carried everywhere, c~ = 2c):
    #   L0 proj (input x, exact):        bank g x2, others x1
    #   L0 rec (input h~0):              x2, bank g x4
    #   L1 proj rows 0:114 (input h~0):  x2, bank g x4; bias row: x1 / g x2
    #   L1 rec (input h~1):              x2, bank g x4
    #   FC (input h~1):                  x2
    bank_s0 = [1.0, 1.0, 1.0, 2.0]  # exact-input projections, per bank
    # ---- L0 projection (+bias via ones row), [208, 4*128] ----
    w0 = np.zeros((DK, 512), f32)
    wf = np.concatenate([inp["wih0f"], inp["b0f"][:, None]], axis=1)  # [200,208]
    wb = np.concatenate([inp["wih0b"], inp["b0b"][:, None]], axis=1)
    for c, gs in enumerate(GATE_SLICES):
        w0[:, c * 128 + 0:c * 128 + 50] = bank_s0[c] * wf[gs].T
        w0[:, c * 128 + Q:c * 128 + Q + 50] = bank_s0[c] * wb[gs].T
    # ---- L0 recurrent, block-diag [114, 4*128] ----
    r0 = np.zeros((KH, 512), f32)
    for c, gs in enumerate(GATE_SLICES):
        s = 2.0 * bank_s0[c]
        r0[0:50, c * 128 + 0:c * 128 + 50] = s * inp["whh0f"][gs].T
        r0[Q:Q + 50, c * 128 + Q:c * 128 + Q + 50] = s * inp["whh0b"][gs].T
    # ---- L1 fwd projection [115, 4*64] (rows: h1f 0:50, h1b 64:114, bias 114)
    def l1_proj(wih, b):
        w = np.zeros((KH1, 256), f32)
        for c, gs in enumerate(GATE_SLICES):
            s = 2.0 * bank_s0[c]
            w[0:50, c * 64:c * 64 + 50] = s * wih[gs, 0:50].T
            w[Q:Q + 50, c * 64:c * 64 + 50] = s * wih[gs, 50:100].T
            w[KH, c * 64:c * 64 + 50] = bank_s0[c] * b[gs]
        return w
    w1 = l1_proj(inp["wih1f"], inp["b1f"])
    w1b = l1_proj(inp["wih1b"], inp["b1b"])
    # ---- L1 recurrent, block-diag per sweep pair [114, 4*128] ----
    r1 = np.zeros((KH, 512), f32)
    for c, gs in enumerate(GATE_SLICES):
        s = 2.0 * bank_s0[c]
        r1[0:50, c * 128 + 0:c * 128 + 50] = s * inp["whh1f"][gs].T
        r1[Q:Q + 50, c * 128 + Q:c * 128 + Q + 50] = s * inp["whh1f"][gs].T
    # ---- FC ----
    wff = np.ascontiguousarray(2.0 * inp["fc_w"][:, 0:50].T)  # [50, 2]
    wfb = np.ascontiguousarray(2.0 * inp["fc_w"][:, 50:100].T)
    bf = ml_dtypes.bfloat16
    return {
        "w0hi": w0[0:128].astype(bf), "w0lo": w0[128:DK].astype(bf),
        "r0": r0.astype(bf), "w1": w1.astype(bf), "w1b": w1b.astype(bf),
        "r1": r1.astype(bf), "wff": wff.astype(bf), "wfb": wfb.astype(bf),
        "ones": np.ones((1, T, 2 * BS), dtype=bf),
    }


def _pack_x(x):
    # x [B, T, 207] f32 -> [T, 208, B] bf16 with ones row at d=207
    xt = np.empty((T, DK, B), dtype=ml_dtypes.bfloat16)
    xt[:, 0:DIN, :] = x.transpose(1, 2, 0).astype(ml_dtypes.bfloat16)
    xt[:, DIN, :] = ml_dtypes.bfloat16(1.0)
    return xt


def _mm(nc, out, lhsT, rhs, start, stop, tp=None):
    kw = {}
    if tp is not None and USE_TP:
        kw["tile_position"] = tp
    nc.tensor.matmul(out, lhsT, rhs, start=start, stop=stop, **kw)


def _build_nc():
    nc = bacc.Bacc("TRN2", target_bir_lowering=False, debug=False)
    ap = {}
    ap["xT"] = nc.dram_tensor("xT", [T, DK, BC], BF16, kind="ExternalInput").ap()
    for name, shp in [("w0hi", [128, 512]), ("w0lo", [DK - 128, 512]),
                      ("r0", [KH, 512]), ("w1", [KH1, 256]), ("w1b", [KH1, 256]),
                      ("r1", [KH, 512]), ("wff", [50, 2]), ("wfb", [50, 2]),
                      ("ones", [1, T, 2 * BS])]:
        ap[name] = nc.dram_tensor(name, shp, BF16, kind="ExternalInput").ap()
    out_ap = nc.dram_tensor("out", [2, BC], F32, kind="ExternalOutput").ap()

    with tile.TileContext(nc) as tc:
        with (
            tc.tile_pool(name="wts", bufs=1) as wp,
            tc.tile_pool(name="xin", bufs=2) as xp,
            tc.tile_pool(name="h1p", bufs=1) as h1p,
            tc.tile_pool(name="st", bufs=1) as sp,
            tc.tile_pool(name="gt", bufs=1) as gp,
            tc.tile_pool(name="outp", bufs=1) as op_,
        ):
            # ---- load weights to SBUF ----
            w = {}
            for name, shp in [("w0hi", [128, 512]), ("w0lo", [DK - 128, 512]),
                              ("r0", [KH, 512]), ("w1", [KH1, 256]),
                              ("w1b", [KH1, 256]), ("r1", [KH, 512]),
                              ("wff", [50, 2]), ("wfb", [50, 2])]:
                t = wp.tile(shp, BF16, tag=name, name=name)
                nc.sync.dma_start(t[:], ap[name][:])
                w[name] = t
            # FC weights staged at both quadrants (rhs base-partition match)
            for name in ("wff", "wfb"):
                t = wp.tile([128, 2], BF16, tag=name + "q", name=name + "q")
                nc.sync.dma_start(t[0:50, :], ap[name][:])
                nc.sync.dma_start(t[Q:Q + 50, :], ap[name][:])
                w[name + "q"] = t

            # ---- h1 history per sweep-PAIR [115, T, 1024]; row 114 = ones
            # (rows 50:64 get finite junk from the widened fwd h1 DMA each
            # step; row 114 is the L1 bias ones row, written once by DMA;
            # rows 115:128 are outside the K span.)
            h1 = []
            for p in range(2):
                t = h1p.tile([128, T, 2 * BS], BF16, tag=f"h1_{p}", name=f"h1_{p}")
                nc.sync.dma_start(t[KH:KH + 1, :, :], ap["ones"][:])
                h1.append(t)

            # ---- states (h~ per sweep-PAIR so h1 stores are 1 DMA/pair) ----
            hS = [sp.tile([128, 2 * BS], BF16, tag=f"hs{p}", name=f"hs{p}") for p in range(2)]
            cS = [sp.tile([128, BS], F32, tag=f"cs{s}", name=f"cs{s}") for s in range(NSW)]
            tS = [sp.tile([128, BS], BF16, tag=f"ts{s}", name=f"ts{s}") for s in range(NSW)]
            # G in F32: (sg-0.5)/(sc-0.5) would cancel catastrophically in
            # bf16 near g=0/c=0; fp32 sigma outputs keep the subtraction
            # exact, bf16 only on t/h~ where error is relative.
            hB = [sp.tile([128, BS], BF16, tag=f"hb{p}", name=f"hb{p}") for p in range(2)]
            cB = [sp.tile([128, BS], F32, tag=f"cb{p}", name=f"cb{p}") for p in range(2)]
            tB = [sp.tile([128, BS], BF16, tag=f"tb{p}", name=f"tb{p}") for p in range(2)]
            hC = [sp.tile([128, BS], BF16, tag=f"hc{p}", name=f"hc{p}") for p in range(2)]
            cC = [sp.tile([128, BS], F32, tag=f"cc{p}", name=f"cc{p}") for p in range(2)]
            tC = [sp.tile([128, BS], BF16, tag=f"tc{p}", name=f"tc{p}") for p in range(2)]

            def cell(P, G, t_, c_t, h_out, k):
                """P: psum [128,4,BS] gates i,f,o,g2; G: sbuf f32 [128,4,BS];
                t_: bf16 [128,BS] scratch; c_t: [128,BS] f32 (holds c~=2c);
                h_out: [128,BS] bf16 dest (rows 0:KH, holds h~=h/2).
                sc = sigma(c~) reuses G slot 0 (si is dead by then).
                The post-sigmoid tail is priority-hoisted so sigma(c~)/h~
                don't queue behind the next sweep's big 4-bank sigmoid."""
                nc.scalar.activation(G[0:KH, :, :], P[0:KH, :, :], AF.Sigmoid)
                si, sf, so, sg = (G[0:KH, j, :] for j in range(4))
                tt = t_[0:KH, :]
                with tc.high_priority(offset=60):
                    # t = (sg - 0.5) * si  == i * tanh(g) / 2
                    nc.vector.scalar_tensor_tensor(tt, sg, 0.5, si,
                                                   ALU.subtract, ALU.mult)
                    if k == 0:
                        nc.vector.tensor_scalar_mul(c_t[0:KH, :], tt, 4.0)
                    else:
                        nc.vector.tensor_mul(c_t[0:KH, :], sf, c_t[0:KH, :])
                        nc.vector.scalar_tensor_tensor(c_t[0:KH, :], tt, 4.0,
                                                       c_t[0:KH, :],
                                                       ALU.mult, ALU.add)
                    # sc = sigma(c~) = (tanh(c)+1)/2 -> reuse si slot
                    nc.scalar.activation(si, c_t[0:KH, :], AF.Sigmoid)
                    # h~ = (sc - 0.5) * so == o * tanh(c) / 2
                    nc.vector.scalar_tensor_tensor(h_out[0:KH, :], si, 0.5, so,
                                                   ALU.subtract, ALU.mult)

            with tc.tile_pool(name="ps", bufs=2, space="PSUM") as pp:
                # =================== Phase A: layer 0, lockstep ===========
                for k in range(T):
                    for p in range(2):
                        psl = bass.ts(p, 2 * BS)
                        # pair-wide x tiles (2 sweeps per DMA): 8 Sync
                        # issues/step instead of 16 - the DMA-issue path on
                        # the Sync queue (~0.7us per DMA_DIRECT2D) was the
                        # phase-A co-bottleneck.
                        xfh = xp.tile([128, 2 * BS], BF16, tag="xfh", name="xfh")
                        nc.sync.dma_start(xfh[:], ap["xT"][k, 0:128, psl])
                        xfl = xp.tile([DK - 128, 2 * BS], BF16, tag="xfl", name="xfl")
                        nc.sync.dma_start(xfl[:], ap["xT"][k, 128:DK, psl])
                        xbh = xp.tile([128, 2 * BS], BF16, tag="xbh", name="xbh")
                        nc.sync.dma_start(xbh[:], ap["xT"][T - 1 - k, 0:128, psl])
                        xbl = xp.tile([DK - 128, 2 * BS], BF16, tag="xbl", name="xbl")
                        nc.sync.dma_start(xbl[:], ap["xT"][T - 1 - k, 128:DK, psl])
                        for j in range(2):
                            s = 2 * p + j
                            sl = bass.ts(j, BS)

                            # Rotated interleave: adjacent MMs hit disjoint
                            # col-groups (concurrent) while same-bank writes
                            # are >=2 slots apart (concurrent drains from
                            # different col-groups into one bank corrupt it).
                            P = pp.tile([128, 4, BS], F32, tag="ps", name="ps")
                            for wt_, xfx, xbx, st in ((w["w0hi"], xfh, xbh, True),
                                                      (w["w0lo"], xfl, xbl, False)):
                                for c in range(4):
                                    cb = (c + 2) % 4
                                    spA = k == 0 and not st and c >= 2
                                    spB = k == 0 and not st and cb < 2
                                    _mm(nc, P[0:Q, c, :],
                                        wt_[:, bass.ds(c * 128, Q)], xfx[:, sl],
                                        start=st, stop=spA)
                                    _mm(nc, P[Q:128, cb, :],
                                        wt_[:, bass.ds(cb * 128 + Q, Q)], xbx[:, sl],
                                        start=st, stop=spB)
                            if k > 0:
                                for c in range(4):
                                    _mm(nc, P[:, c, :],
                                        w["r0"][:, bass.ts(c, 128)],
                                        hS[p][0:KH, sl],
                                        start=False, stop=True)

                            G = gp.tile([128, 4, BS], F32, tag=f"g{s}", name=f"g{s}")
                            cell(P, G, tS[s], cS[s], hS[p][:, sl], k)
                        # h1 history is time-aligned: bwd lane at step k
                        # holds h_b for time T-1-k. Fwd store widened to
                        # 0:64 so junk rows 50:64 hold finite data. One
                        # DMA per pair (both sweeps' h~ contiguous in hS).
                        nc.sync.dma_start(h1[p][0:Q, k, :], hS[p][0:Q, :])
                        nc.sync.dma_start(h1[p][Q:Q + 50, T - 1 - k, :],
                                          hS[p][Q:Q + 50, :])

                # ============ Phase C: layer 1 backward (t=29) ============
                # Emitted BEFORE phase B: C depends only on phase A's h1, so
                # its two groups fill phase B's chain-bound startup bubbles.
                for p in range(2):
                    h1a = h1[p][0:KH1, T - 1, 0:BS]
                    h1b = h1[p][0:KH1, T - 1, BS:2 * BS]
                    P = pp.tile([128, 4, BS], F32, tag="ps", name="ps")
                    for c in range(4):
                        cb = (c + 2) % 4
                        _mm(nc, P[0:Q, c, :], w["w1b"][:, bass.ts(c, 64)],
                            h1a, start=True, stop=c >= 2)
                        _mm(nc, P[Q:128, cb, :], w["w1b"][:, bass.ts(cb, 64)],
                            h1b, start=True, stop=cb < 2)
                    G = gp.tile([128, 4, BS], F32, tag=f"g{p}", name=f"g{p}")
                    cell(P, G, tC[p], cC[p], hC[p], 0)

                # =================== Phase B: layer 1 forward =============
                for k in range(T):
                    for p in range(2):
                        h1a = h1[p][0:KH1, k, 0:BS]
                        h1b = h1[p][0:KH1, k, BS:2 * BS]
                        P = pp.tile([128, 4, BS], F32, tag="ps", name="ps")
                        for c in range(4):
                            cb = (c + 2) % 4
                            _mm(nc, P[0:Q, c, :], w["w1"][:, bass.ts(c, 64)],
                                h1a, start=True,
                                stop=(k == 0 and c >= 2))
                            _mm(nc, P[Q:128, cb, :], w["w1"][:, bass.ts(cb, 64)],
                                h1b, start=True,
                                stop=(k == 0 and cb < 2))
                        if k > 0:
                            for c in range(4):
                                _mm(nc, P[:, c, :],
                                    w["r1"][:, bass.ts(c, 128)], hB[p][0:KH, :],
                                    start=False, stop=True)
                        G = gp.tile([128, 4, BS], F32, tag=f"g{p}", name=f"g{p}")
                        cell(P, G, tB[p], cB[p], hB[p], k)

            # =================== FC ====================================
            with tc.tile_pool(name="fcp", bufs=4, space="PSUM") as fcp:
                for s in range(NSW):
                    p, qo = s // 2, (s % 2) * Q
                    F = fcp.tile([2, BS], F32, tag="fc", name="fc")
                    nc.tensor.matmul(F[:], w["wffq"][qo:qo + 50, :],
                                     hB[p][qo:qo + 50, :], start=True, stop=False)
                    nc.tensor.matmul(F[:], w["wfbq"][qo:qo + 50, :],
                                     hC[p][qo:qo + 50, :], start=False, stop=True)
                    ot = op_.tile([2, BS], F32, tag="o", name="ot")
                    nc.vector.tensor_copy(ot[:], F[:])
                    nc.sync.dma_start(out_ap[:, bass.ts(s, BS)], ot[:])

    nc.compile()
    return nc


_NC_CACHE = None


def kernel(**inputs) -> np.ndarray:
    global _NC_CACHE
    if _NC_CACHE is None:
        _NC_CACHE = _build_nc()
    nc = _NC_CACHE
    wts = _pack_weights(inputs)
    xt = _pack_x(np.asarray(inputs["x"], dtype=np.float32))
    in_maps = []
    for c in range(NCORES):
        m = dict(wts)
        m["xT"] = np.ascontiguousarray(xt[:, :, c * BC:(c + 1) * BC])
        in_maps.append(m)
    res = run_bass_kernel_spmd(nc, in_maps, list(range(NCORES)))
    outs = [res.results[c]["out"] for c in range(NCORES)]  # [2, BC] each
    full = np.concatenate(outs, axis=1).T  # [B, 2]
    return (full + inputs["fc_b"][None, :]).astype(np.float32)
